# revision 24
# baseline (speedup 1.0000x reference)
"""AverageTokenDownsampler (segment-mean) Trainium2 kernel.

Strategy (data-parallel over batch, 1 row per NeuronCore):
  Per batch row b:  x_down[m] = mean_{s: dst[s]==m} x[s],
                    p_down[m] = floor(sum pos[s] / cnt[m])  (0 when empty).

  dst is SORTED per row, so tokens mapping to an aligned 128-wide block of
  destinations k (m in [128k, 128k+128)) come from a small contiguous set of
  128-token tiles t. We compute, on host, the union (over the 8 rows) of
  (t, k) pairs whose dst-ranges intersect, and emit one static SPMD program:

    for k:  for t in T_k:
        A_t[s, m'] = (dst[s] == 128*kf(t) + m')   # one-hot vs wide iota, DVE
        psum_x[m, :]   += A_t[:, k-slice].T @ x_tile      # TensorE, bf16
        psum_aux[m, :] += A_t[:, k-slice].T @ [1,hi,lo]   # pos exact in bf16
    x_down_block = psum_x * (1/max(cnt,1));  p_down = floor-div, batched

  Pairs not needed by a given row produce all-zero one-hot columns and
  contribute nothing, so one program is correct for all 8 cores (~62 pairs
  vs 32*16=512 dense).

  DMA layout: descriptors are per-SBUF-partition, so per-partition DRAM
  contiguity sets packet size. x is fed host-permuted as (128, 32, 1024)
  [p, t, d] = x[128t+p, d], giving multi-KB/partition descriptors (vs 4 KB
  token-major, which runs ~3x slower); the input DMA casts f32->bf16 inline
  (SWDGE), and outputs are written in analogous permuted layouts in batches
  and unscrambled on host.
"""

import sys

import numpy as np

for _p in ("/opt/trn_rl_repo", "/root/.axon_site/_ro/trn_rl_repo"):
    if _p not in sys.path:
        sys.path.insert(0, _p)

P = 128          # partitions / tile edge
B, S, D = 8, 4096, 1024
M = 2048         # max_n_dst
NT = S // P      # 32 s-tiles per row
NK = M // P      # 16 m-blocks per row
N_CORES = 8
CHUNK = 4        # s-tiles per x staging DMA (16KB/partition descriptors)
OBATCH = 4       # k-blocks per output DMA (16KB/partition descriptors)

LAST_RESULTS = None  # BassKernelResults of the most recent run (for test.py)


def _band_structure(dst: np.ndarray) -> list[list[int]]:
    """T_k[k] = sorted s-tile indices whose dst-range intersects block k,
    unioned over all batch rows. dst: (B, S) sorted int32."""
    needed = set()
    for b in range(dst.shape[0]):
        row = dst[b]
        for t in range(NT):
            k_lo = int(row[P * t]) // P
            k_hi = int(row[P * t + P - 1]) // P
            for k in range(k_lo, k_hi + 1):
                needed.add((t, k))
    T_k = [[] for _ in range(NK)]
    for t, k in sorted(needed):
        T_k[k].append(t)
    return T_k


def _k_ranges(T_k: list[list[int]]) -> dict[int, tuple[int, int]]:
    """t -> (k_first, k_last) over the union band structure."""
    rng = {}
    for k, ts in enumerate(T_k):
        for t in ts:
            if t in rng:
                rng[t] = (rng[t][0], k)
            else:
                rng[t] = (k, k)
    return rng


def _bufs_needed(T_k: list[list[int]], group: int) -> int:
    """Max live slots if units of `group` s-tiles are allocated at first use
    (k ascending) and freed after their last use."""
    first_k, last_k = {}, {}
    for k, ts in enumerate(T_k):
        for t in ts:
            u = t // group
            first_k.setdefault(u, k)
            last_k[u] = k
    max_live = 0
    for k in range(NK):
        live = sum(1 for u in first_k if first_k[u] <= k <= last_k[u])
        max_live = max(max_live, live)
    return max_live


def _build_program(T_k: list[list[int]]):
    import concourse.bacc as bacc
    import concourse.tile as tile
    from concourse import mybir

    f32 = mybir.dt.float32
    bf16 = mybir.dt.bfloat16
    i32 = mybir.dt.int32
    Alu = mybir.AluOpType

    nc = bacc.Bacc("TRN2", target_bir_lowering=False, debug=False,
                   num_devices=N_CORES)

    x_in = nc.dram_tensor("x", [P, NT, D], f32, kind="ExternalInput")
    dst_in = nc.dram_tensor("dst", [P, NT], i32, kind="ExternalInput")
    pos_in = nc.dram_tensor("pos", [P, NT], i32, kind="ExternalInput")
    out_x = nc.dram_tensor("out_x", [P, NK, D], f32, kind="ExternalOutput")
    out_p = nc.dram_tensor("out_p", [P, NK], i32, kind="ExternalOutput")

    kr = _k_ranges(T_k)
    max_span = max(k1 - k0 + 1 for k0, k1 in kr.values())
    x_bufs = _bufs_needed(T_k, 1) + 2        # live one-hot tiles
    c_bufs = max(3, _bufs_needed(T_k, CHUNK) + 1)  # live bf16 chunks

    with tile.TileContext(nc) as tc:
        with (
            tc.tile_pool(name="const", bufs=1) as const_pool,
            tc.tile_pool(name="xstage", bufs=c_bufs) as xs_pool,
            tc.tile_pool(name="amat", bufs=x_bufs) as a_pool,
            tc.tile_pool(name="small", bufs=8) as small_pool,
            tc.tile_pool(name="xout", bufs=2) as out_pool,
            tc.tile_pool(name="psx", bufs=3, space="PSUM") as psx_pool,
            tc.tile_pool(name="psa", bufs=2, space="PSUM") as psa_pool,
        ):
            # bf16 x chunks: SWDGE (gpsimd) DMA casts f32->bf16 inline,
            # loaded on demand (prefetching everything overlaps the reads
            # with the output writes, which measures slower on HBM)
            chunks = {}

            def get_chunk(c):
                if c in chunks:
                    return chunks[c]
                ch = xs_pool.tile([P, CHUNK, D], bf16, tag="ch")
                nc.gpsimd.dma_start(ch[:],
                                    x_in[:, CHUNK * c:CHUNK * (c + 1), :])
                chunks[c] = ch
                return ch

            # constants / metadata (loaded once)
            iota_i = const_pool.tile([P, P * max_span], i32, tag="iota_i")
            nc.gpsimd.iota(iota_i[:], pattern=[[1, P * max_span]], base=0,
                           channel_multiplier=0)
            iota_f = const_pool.tile([P, P * max_span], f32, tag="iota_f")
            nc.vector.tensor_copy(iota_f[:], iota_i[:])

            dst_i = const_pool.tile([P, NT], i32, tag="dst_i")
            nc.sync.dma_start(dst_i[:], dst_in[:, :])
            dst_f = const_pool.tile([P, NT], f32, tag="dst_f")
            nc.vector.tensor_copy(dst_f[:], dst_i[:])

            # aux rhs table: [:, t, :] = [1.0, pos_hi, pos_lo] for tile t,
            # with pos = 128*hi + lo split exactly into bf16
            pos_i = const_pool.tile([P, NT], i32, tag="pos_i")
            nc.sync.dma_start(pos_i[:], pos_in[:, :])
            hi_i = const_pool.tile([P, NT], i32, tag="hi_i")
            nc.vector.tensor_scalar(hi_i[:], pos_i[:], 7, None,
                                    Alu.arith_shift_right)
            lo_i = const_pool.tile([P, NT], i32, tag="lo_i")
            nc.vector.tensor_scalar(lo_i[:], pos_i[:], 127, None,
                                    Alu.bitwise_and)
            aux3 = const_pool.tile([P, NT, 3], bf16, tag="aux3")
            nc.vector.memset(aux3[:, :, 0:1], 1.0)
            nc.vector.tensor_copy(aux3[:, :, 1], hi_i[:])
            nc.vector.tensor_copy(aux3[:, :, 2], lo_i[:])

            # aux sums land here per k ([:, 0, k]=cnt, [:, 1, k]=hi,
            # [:, 2, k]=lo); p_down floor math is batched at the end
            aux_all = const_pool.tile([P, 3, NK], f32, tag="aux_all")
            p_all = const_pool.tile([P, NK], i32, tag="p_all")

            # per-tile wide one-hots A_t (DVE), built at first use
            a_tiles = {}

            def get_x(t):
                ch = get_chunk(t // CHUNK)
                xt = ch[:, t % CHUNK, :]
                if t in a_tiles:
                    return xt, a_tiles[t]
                k0, k1 = kr[t]
                span = k1 - k0 + 1
                At = a_pool.tile([P, P * span], bf16, tag="A")
                # A_t[s, j] = (iota[j] + 128*k0 == dst[s])
                nc.vector.tensor_scalar(
                    At[:], iota_f[:, 0:P * span], float(P * k0),
                    dst_f[:, t:t + 1], Alu.add, Alu.is_equal)
                a_tiles[t] = At
                return xt, At

            obatches = []
            kk = 0
            while kk < NK:
                n = min(OBATCH, NK - kk)
                if NK - kk <= OBATCH:          # split the tail for overlap
                    n = max(1, (NK - kk) // 2)
                obatches.append((kk, kk + n))
                kk += n
            ob_of_k = {}
            for b0, b1 in obatches:
                for k in range(b0, b1):
                    ob_of_k[k] = (b0, b1)

            xo_big = None
            for k in range(NK):
                b0, b1 = ob_of_k[k]
                if k == b0:
                    xo_big = out_pool.tile([P, b1 - b0, D], f32, tag="xo")
                oq = k - b0
                ts = T_k[k]
                if not ts:
                    nc.vector.memset(xo_big[:, oq, :], 0.0)
                    nc.vector.memset(aux_all[:, :, k:k + 1], 0.0)
                else:
                    psx = psx_pool.tile([P, D], f32, tag="psx")
                    psa = psa_pool.tile([P, 3], f32, tag="psa")
                    for i, t in enumerate(ts):
                        xt, At = get_x(t)
                        k0 = kr[t][0]
                        Ak = At[:, P * (k - k0):P * (k - k0 + 1)]
                        st, sp = (i == 0), (i == len(ts) - 1)
                        nc.tensor.matmul(psx[:, 0:512], Ak, xt[:, 0:512],
                                         start=st, stop=sp)
                        nc.tensor.matmul(psx[:, 512:1024], Ak,
                                         xt[:, 512:1024], start=st, stop=sp)
                        nc.tensor.matmul(psa[:, 0:3], Ak, aux3[:, t, :],
                                         start=st, stop=sp)

                    # epilogue: x_down = psx / max(cnt, 1)
                    safe = small_pool.tile([P, 1], f32, tag="safe")
                    nc.vector.tensor_scalar_max(safe[:], psa[:, 0:1], 1.0)
                    inv = small_pool.tile([P, 1], f32, tag="inv")
                    nc.vector.reciprocal(inv[:], safe[:])
                    nc.scalar.activation(
                        xo_big[:, oq, :], psx[:],
                        mybir.ActivationFunctionType.Copy, scale=inv[:])
                    nc.scalar.copy(aux_all[:, :, k:k + 1], psa[:, 0:3])

                if k == b1 - 1:
                    nc.sync.dma_start(out_x[:, b0:b1, :], xo_big[:])

            # batched p_down: q = (128*hi + lo + 0.5) / max(cnt,1);
            # p_down = floor(q) via f32->i32 convert + correction (works
            # whether the convert truncates or rounds-to-nearest)
            cnt_a = aux_all[:, 0, :]
            hi_a = aux_all[:, 1, :]
            lo_a = aux_all[:, 2, :]
            safe_a = const_pool.tile([P, NK], f32, tag="safe_a")
            nc.vector.tensor_scalar_max(safe_a[:], cnt_a, 1.0)
            inv_a = const_pool.tile([P, NK], f32, tag="inv_a")
            nc.vector.reciprocal(inv_a[:], safe_a[:])
            q = const_pool.tile([P, NK], f32, tag="q")
            nc.vector.tensor_scalar(q[:], hi_a, 128.0, 0.5, Alu.mult, Alu.add)
            nc.vector.tensor_add(q[:], q[:], lo_a)
            nc.vector.tensor_mul(q[:], q[:], inv_a[:])
            ci = const_pool.tile([P, NK], i32, tag="ci")
            nc.vector.tensor_copy(ci[:], q[:])
            cf = const_pool.tile([P, NK], f32, tag="cf")
            nc.vector.tensor_copy(cf[:], ci[:])
            gt = const_pool.tile([P, NK], f32, tag="gt")
            nc.vector.tensor_tensor(gt[:], cf[:], q[:], Alu.is_gt)
            nc.vector.tensor_sub(cf[:], cf[:], gt[:])
            nc.vector.tensor_copy(p_all[:], cf[:])
            nc.sync.dma_start(out_p[:, :], p_all[:])

    nc.compile()
    return nc


def _ensure_axon_hooks_module():
    try:
        import antenv.axon_hooks  # noqa: F401
        return
    except ImportError:
        pass
    import types
    try:
        import antenv  # noqa: F401
    except ImportError:
        sys.modules.setdefault("antenv", types.ModuleType("antenv"))
    mod = types.ModuleType("antenv.axon_hooks")
    _hook = [None]
    mod.set_axon_ntff_profile_hook = lambda h: _hook.__setitem__(0, h)
    mod.get_axon_ntff_profile_hook = lambda: _hook[0]
    sys.modules["antenv.axon_hooks"] = mod


_PROGRAM_CACHE = {}


def kernel(x, position_ids, down_merge_dst, max_n_dst):
    global LAST_RESULTS
    _ensure_axon_hooks_module()
    from concourse.bass_utils import run_bass_kernel_spmd

    x = np.asarray(x, dtype=np.float32)
    pos = np.ascontiguousarray(np.asarray(position_ids, dtype=np.int32))
    dst = np.ascontiguousarray(np.asarray(down_merge_dst, dtype=np.int32))
    assert x.shape == (B, S, D) and dst.shape == (B, S)
    assert int(max_n_dst) == M

    T_k = _band_structure(dst)
    cache_key = tuple(tuple(ts) for ts in T_k)
    nc = _PROGRAM_CACHE.get(cache_key)
    if nc is None:
        nc = _build_program(T_k)
        _PROGRAM_CACHE[cache_key] = nc

    in_maps = []
    for c in range(N_CORES):
        in_maps.append({
            # [p, t, d] = x[c, 128*t + p, d]: 32KB contiguous per partition
            "x": np.ascontiguousarray(
                x[c].reshape(NT, P, D).transpose(1, 0, 2)),
            # partition-major layout: [p, t] = value at token 128*t + p
            "dst": np.ascontiguousarray(dst[c].reshape(NT, P).T),
            "pos": np.ascontiguousarray(pos[c].reshape(NT, P).T),
        })

    res = run_bass_kernel_spmd(nc, in_maps, core_ids=list(range(N_CORES)))
    LAST_RESULTS = res

    # out_x[p, k, d] = x_down[128k + p, d]; out_p[p, k] = p_down[128k + p]
    x_down = np.stack([
        res.results[c]["out_x"].transpose(1, 0, 2).reshape(M, D)
        for c in range(N_CORES)
    ])
    p_down = np.stack([
        res.results[c]["out_p"].T.reshape(M) for c in range(N_CORES)
    ])
    return x_down.astype(np.float32), p_down.astype(np.int32)


# revision 32
# speedup vs baseline: 1.1705x; 1.1705x over previous
"""AverageTokenDownsampler (segment-mean) Trainium2 kernel.

Strategy (data-parallel over batch, 1 row per NeuronCore):
  Per batch row b:  x_down[m] = mean_{s: dst[s]==m} x[s],
                    p_down[m] = floor(sum pos[s] / cnt[m])  (0 when empty).

  dst is SORTED per row, so tokens mapping to an aligned 128-wide block of
  destinations k (m in [128k, 128k+128)) come from a small contiguous set of
  128-token tiles t. We compute, on host, the union (over the 8 rows) of
  (t, k) pairs whose dst-ranges intersect, and emit one static SPMD program:

    for k:  for t in T_k:
        A_t[s, m'] = (dst[s] == 128*kf(t) + m')   # one-hot vs wide iota, DVE
        psum_x[m, :]   += A_t[:, k-slice].T @ x_tile      # TensorE, bf16
        psum_aux[m, :] += A_t[:, k-slice].T @ [1,hi,lo]   # pos exact in bf16
    x_down_block = psum_x * (1/max(cnt,1));  p_down = floor-div, batched

  Pairs not needed by a given row produce all-zero one-hot columns and
  contribute nothing, so one program is correct for all 8 cores (~62 pairs
  vs 32*16=512 dense).

  DMA layout: descriptors are per-SBUF-partition, so per-partition DRAM
  contiguity sets packet size. x is fed host-permuted as (128, 32, 1024)
  [p, t, d] = x[128t+p, d], giving multi-KB/partition descriptors (vs 4 KB
  token-major, which runs ~3x slower); the input DMA casts f32->bf16 inline
  (SWDGE), and outputs are written in analogous permuted layouts in batches
  and unscrambled on host.
"""

import sys

import numpy as np

for _p in ("/opt/trn_rl_repo", "/root/.axon_site/_ro/trn_rl_repo"):
    if _p not in sys.path:
        sys.path.insert(0, _p)

P = 128          # partitions / tile edge
B, S, D = 8, 4096, 1024
M = 2048         # max_n_dst
NT = S // P      # 32 s-tiles per row
NK = M // P      # 16 m-blocks per row
N_CORES = 8
CHUNK = 4        # s-tiles per x staging DMA (16KB/partition descriptors)
OBATCH = 8       # k-blocks per output DMA (16KB/partition bf16 descriptors)

LAST_RESULTS = None  # BassKernelResults of the most recent run (for test.py)


def _band_structure(dst: np.ndarray) -> list[list[int]]:
    """T_k[k] = sorted s-tile indices whose dst-range intersects block k,
    unioned over all batch rows. dst: (B, S) sorted int32."""
    needed = set()
    for b in range(dst.shape[0]):
        row = dst[b]
        for t in range(NT):
            k_lo = int(row[P * t]) // P
            k_hi = int(row[P * t + P - 1]) // P
            for k in range(k_lo, k_hi + 1):
                needed.add((t, k))
    T_k = [[] for _ in range(NK)]
    for t, k in sorted(needed):
        T_k[k].append(t)
    return T_k


def _k_ranges(T_k: list[list[int]]) -> dict[int, tuple[int, int]]:
    """t -> (k_first, k_last) over the union band structure."""
    rng = {}
    for k, ts in enumerate(T_k):
        for t in ts:
            if t in rng:
                rng[t] = (rng[t][0], k)
            else:
                rng[t] = (k, k)
    return rng


def _bufs_needed(T_k: list[list[int]], group: int) -> int:
    """Max live slots if units of `group` s-tiles are allocated at first use
    (k ascending) and freed after their last use."""
    first_k, last_k = {}, {}
    for k, ts in enumerate(T_k):
        for t in ts:
            u = t // group
            first_k.setdefault(u, k)
            last_k[u] = k
    max_live = 0
    for k in range(NK):
        live = sum(1 for u in first_k if first_k[u] <= k <= last_k[u])
        max_live = max(max_live, live)
    return max_live


def _build_program(T_k: list[list[int]]):
    import concourse.bacc as bacc
    import concourse.tile as tile
    from concourse import mybir

    f32 = mybir.dt.float32
    bf16 = mybir.dt.bfloat16
    i32 = mybir.dt.int32
    Alu = mybir.AluOpType

    nc = bacc.Bacc("TRN2", target_bir_lowering=False, debug=False,
                   num_devices=N_CORES)

    x_in = nc.dram_tensor("x", [P, NT, D], f32, kind="ExternalInput")
    dst_in = nc.dram_tensor("dst", [P, NT], i32, kind="ExternalInput")
    pos_in = nc.dram_tensor("pos", [P, NT], i32, kind="ExternalInput")
    # x_down is computed in bf16 anyway; storing it as bf16 halves the
    # output HBM traffic (host upcasts to f32)
    out_x = nc.dram_tensor("out_x", [P, NK, D], bf16, kind="ExternalOutput")
    out_p = nc.dram_tensor("out_p", [P, NK], i32, kind="ExternalOutput")

    kr = _k_ranges(T_k)
    max_span = max(k1 - k0 + 1 for k0, k1 in kr.values())
    # chunk plan: tile-range per input DMA; a small first chunk gets the
    # stream flowing sooner (less Q7 descriptor-gen before first bytes)
    chunk_bounds = [0, 2]
    while chunk_bounds[-1] < NT:
        chunk_bounds.append(min(NT, chunk_bounds[-1] + CHUNK))
    chunk_of_t = {}
    for ci in range(len(chunk_bounds) - 1):
        for t in range(chunk_bounds[ci], chunk_bounds[ci + 1]):
            chunk_of_t[t] = ci
    x_bufs = _bufs_needed(T_k, 1) + 2        # live one-hot tiles
    c_bufs = 4  # live bf16 chunk window (chunks are <= CHUNK tiles each)

    with tile.TileContext(nc) as tc:
        with (
            tc.tile_pool(name="const", bufs=1) as const_pool,
            tc.tile_pool(name="xstage", bufs=c_bufs) as xs_pool,
            tc.tile_pool(name="amat", bufs=x_bufs) as a_pool,
            tc.tile_pool(name="small", bufs=8) as small_pool,
            tc.tile_pool(name="xout", bufs=2) as out_pool,
            tc.tile_pool(name="psx", bufs=3, space="PSUM") as psx_pool,
            tc.tile_pool(name="psa", bufs=2, space="PSUM") as psa_pool,
        ):
            # bf16 x chunks: SWDGE (gpsimd) DMA casts f32->bf16 inline,
            # loaded on demand (prefetching everything overlaps the reads
            # with the output writes, which measures slower on HBM)
            chunks = {}

            def get_chunk(c):
                if c in chunks:
                    return chunks[c]
                t0, t1 = chunk_bounds[c], chunk_bounds[c + 1]
                ch = xs_pool.tile([P, t1 - t0, D], bf16, tag="ch")
                nc.gpsimd.dma_start(ch[:], x_in[:, t0:t1, :])
                chunks[c] = ch
                return ch

            # constants / metadata (loaded once)
            iota_i = const_pool.tile([P, P * max_span], i32, tag="iota_i")
            nc.gpsimd.iota(iota_i[:], pattern=[[1, P * max_span]], base=0,
                           channel_multiplier=0)
            iota_f = const_pool.tile([P, P * max_span], f32, tag="iota_f")
            nc.vector.tensor_copy(iota_f[:], iota_i[:])

            dst_i = const_pool.tile([P, NT], i32, tag="dst_i")
            nc.sync.dma_start(dst_i[:], dst_in[:, :])
            dst_f = const_pool.tile([P, NT], f32, tag="dst_f")
            nc.vector.tensor_copy(dst_f[:], dst_i[:])

            # aux rhs table: [:, t, :] = [1.0, pos_hi, pos_lo] for tile t,
            # with pos = 128*hi + lo split exactly into bf16
            pos_i = const_pool.tile([P, NT], i32, tag="pos_i")
            nc.sync.dma_start(pos_i[:], pos_in[:, :])
            hi_i = const_pool.tile([P, NT], i32, tag="hi_i")
            nc.vector.tensor_scalar(hi_i[:], pos_i[:], 7, None,
                                    Alu.arith_shift_right)
            lo_i = const_pool.tile([P, NT], i32, tag="lo_i")
            nc.vector.tensor_scalar(lo_i[:], pos_i[:], 127, None,
                                    Alu.bitwise_and)
            aux3 = const_pool.tile([P, NT, 3], bf16, tag="aux3")
            nc.vector.memset(aux3[:, :, 0:1], 1.0)
            nc.vector.tensor_copy(aux3[:, :, 1], hi_i[:])
            nc.vector.tensor_copy(aux3[:, :, 2], lo_i[:])

            # aux sums land here per k ([:, 0, k]=cnt, [:, 1, k]=hi,
            # [:, 2, k]=lo); p_down floor math is batched at the end
            aux_all = const_pool.tile([P, 3, NK], f32, tag="aux_all")
            p_all = const_pool.tile([P, NK], i32, tag="p_all")

            # per-tile wide one-hots A_t (DVE), built at first use
            a_tiles = {}

            def get_x(t):
                c = chunk_of_t[t]
                ch = get_chunk(c)
                xt = ch[:, t - chunk_bounds[c], :]
                if t in a_tiles:
                    return xt, a_tiles[t]
                k0, k1 = kr[t]
                span = k1 - k0 + 1
                At = a_pool.tile([P, P * span], bf16, tag="A")
                # A_t[s, j] = (iota[j] + 128*k0 == dst[s])
                nc.vector.tensor_scalar(
                    At[:], iota_f[:, 0:P * span], float(P * k0),
                    dst_f[:, t:t + 1], Alu.add, Alu.is_equal)
                a_tiles[t] = At
                return xt, At

            obatches = []
            kk = 0
            while kk < NK:
                n = min(OBATCH, NK - kk)
                if NK - kk <= OBATCH:          # split the tail for overlap
                    n = max(1, (NK - kk) // 2)
                obatches.append((kk, kk + n))
                kk += n
            ob_of_k = {}
            for b0, b1 in obatches:
                for k in range(b0, b1):
                    ob_of_k[k] = (b0, b1)

            xo_big = None
            for k in range(NK):
                b0, b1 = ob_of_k[k]
                if k == b0:
                    xo_big = out_pool.tile([P, b1 - b0, D], bf16, tag="xo")
                oq = k - b0
                ts = T_k[k]
                if not ts:
                    nc.vector.memset(xo_big[:, oq, :], 0.0)
                    nc.vector.memset(aux_all[:, :, k:k + 1], 0.0)
                else:
                    psx = psx_pool.tile([P, D], f32, tag="psx")
                    psa = psa_pool.tile([P, 3], f32, tag="psa")
                    for i, t in enumerate(ts):
                        xt, At = get_x(t)
                        k0 = kr[t][0]
                        Ak = At[:, P * (k - k0):P * (k - k0 + 1)]
                        st, sp = (i == 0), (i == len(ts) - 1)
                        nc.tensor.matmul(psx[:, 0:512], Ak, xt[:, 0:512],
                                         start=st, stop=sp)
                        nc.tensor.matmul(psx[:, 512:1024], Ak,
                                         xt[:, 512:1024], start=st, stop=sp)
                        nc.tensor.matmul(psa[:, 0:3], Ak, aux3[:, t, :],
                                         start=st, stop=sp)

                    # epilogue: x_down = psx / max(cnt, 1)
                    safe = small_pool.tile([P, 1], f32, tag="safe")
                    nc.vector.tensor_scalar_max(safe[:], psa[:, 0:1], 1.0)
                    inv = small_pool.tile([P, 1], f32, tag="inv")
                    nc.vector.reciprocal(inv[:], safe[:])
                    nc.scalar.activation(
                        xo_big[:, oq, :], psx[:],
                        mybir.ActivationFunctionType.Copy, scale=inv[:])
                    nc.scalar.copy(aux_all[:, :, k:k + 1], psa[:, 0:3])

                if k == b1 - 1:
                    nc.sync.dma_start(out_x[:, b0:b1, :], xo_big[:])

            # batched p_down: q = (128*hi + lo + 0.5) / max(cnt,1);
            # p_down = floor(q) via f32->i32 convert + correction (works
            # whether the convert truncates or rounds-to-nearest)
            cnt_a = aux_all[:, 0, :]
            hi_a = aux_all[:, 1, :]
            lo_a = aux_all[:, 2, :]
            safe_a = const_pool.tile([P, NK], f32, tag="safe_a")
            nc.vector.tensor_scalar_max(safe_a[:], cnt_a, 1.0)
            inv_a = const_pool.tile([P, NK], f32, tag="inv_a")
            nc.vector.reciprocal(inv_a[:], safe_a[:])
            q = const_pool.tile([P, NK], f32, tag="q")
            nc.vector.tensor_scalar(q[:], hi_a, 128.0, 0.5, Alu.mult, Alu.add)
            nc.vector.tensor_add(q[:], q[:], lo_a)
            nc.vector.tensor_mul(q[:], q[:], inv_a[:])
            ci = const_pool.tile([P, NK], i32, tag="ci")
            nc.vector.tensor_copy(ci[:], q[:])
            cf = const_pool.tile([P, NK], f32, tag="cf")
            nc.vector.tensor_copy(cf[:], ci[:])
            gt = const_pool.tile([P, NK], f32, tag="gt")
            nc.vector.tensor_tensor(gt[:], cf[:], q[:], Alu.is_gt)
            nc.vector.tensor_sub(cf[:], cf[:], gt[:])
            nc.vector.tensor_copy(p_all[:], cf[:])
            nc.sync.dma_start(out_p[:, :], p_all[:])

    nc.compile()
    return nc


def _ensure_axon_hooks_module():
    try:
        import antenv.axon_hooks  # noqa: F401
        return
    except ImportError:
        pass
    import types
    try:
        import antenv  # noqa: F401
    except ImportError:
        sys.modules.setdefault("antenv", types.ModuleType("antenv"))
    mod = types.ModuleType("antenv.axon_hooks")
    _hook = [None]
    mod.set_axon_ntff_profile_hook = lambda h: _hook.__setitem__(0, h)
    mod.get_axon_ntff_profile_hook = lambda: _hook[0]
    sys.modules["antenv.axon_hooks"] = mod


_PROGRAM_CACHE = {}


def kernel(x, position_ids, down_merge_dst, max_n_dst):
    global LAST_RESULTS
    _ensure_axon_hooks_module()
    from concourse.bass_utils import run_bass_kernel_spmd

    x = np.asarray(x, dtype=np.float32)
    pos = np.ascontiguousarray(np.asarray(position_ids, dtype=np.int32))
    dst = np.ascontiguousarray(np.asarray(down_merge_dst, dtype=np.int32))
    assert x.shape == (B, S, D) and dst.shape == (B, S)
    assert int(max_n_dst) == M

    T_k = _band_structure(dst)
    cache_key = tuple(tuple(ts) for ts in T_k)
    nc = _PROGRAM_CACHE.get(cache_key)
    if nc is None:
        nc = _build_program(T_k)
        _PROGRAM_CACHE[cache_key] = nc

    in_maps = []
    for c in range(N_CORES):
        in_maps.append({
            # [p, t, d] = x[c, 128*t + p, d]: 32KB contiguous per partition
            "x": np.ascontiguousarray(
                x[c].reshape(NT, P, D).transpose(1, 0, 2)),
            # partition-major layout: [p, t] = value at token 128*t + p
            "dst": np.ascontiguousarray(dst[c].reshape(NT, P).T),
            "pos": np.ascontiguousarray(pos[c].reshape(NT, P).T),
        })

    res = run_bass_kernel_spmd(nc, in_maps, core_ids=list(range(N_CORES)))
    LAST_RESULTS = res

    # out_x[p, k, d] = x_down[128k + p, d]; out_p[p, k] = p_down[128k + p]
    x_down = np.stack([
        np.asarray(res.results[c]["out_x"]).astype(np.float32)
        .transpose(1, 0, 2).reshape(M, D)
        for c in range(N_CORES)
    ])
    p_down = np.stack([
        res.results[c]["out_p"].T.reshape(M) for c in range(N_CORES)
    ])
    return x_down.astype(np.float32), p_down.astype(np.int32)
